# revision 33
# baseline (speedup 1.0000x reference)
"""Causal multi-head flash-attention block (QKV proj + attention + out proj)
for Trainium2, distributed over 8 NeuronCores.

Sharding: data-parallel over batch (B=4) x tensor-parallel over head groups
(16 heads -> 2 groups of 8). Core c handles batch c//2, head group c%2.
Each core computes a partial output projection (its 8 heads' contribution);
the host sums the two partials per batch and adds the bias.

v3 schedule notes: the attention inner loop is ACT(exp)-bound, and the PE
executes its queue strictly in order, so independent matmul work is
interleaved INTO the attention k-block loop to keep the PE dense:
  - each pair's (st2,st3) QT/KT units fill its own j0/j1, the NEXT pair's
    (st0,st1) units fill j2/j3 (so casts land well before the pair switch),
  - V strips fill pair 0, out-proj chunks fill pair 3's late slots,
  - inputs arrive via 13 grouped multi-chunk DMAs (3D access patterns) so
    the sync queue isn't issue-bound; pair-0 st0/st1 QT/KT runs tile-minor
    over the arriving chunk-pairs (PE dense from ~11us).
Scores/exp are k-block granular: one ACTIVATE per k-block covering both
packed heads, restricted to causally live columns on diagonal blocks.
The softmax-normalize chain is split in two phases and issues its DMAs
from the producer engines' own queues (dens from vector, the oth
partition-shift from gpsimd) so no FIFO head-of-line-blocks another
engine's PE-feeding work; the multiplies run late (deferred one q-tile)
on the DVE after the gpsimd broadcast is long done.

Per-core kernel (all matmuls bf16 operands, fp32 PSUM accumulate):
  - QKV proj from host-pretransposed x^T: Q^T,K^T in [d, s] layout, V in
    [s, d] layout with a ones-column per head (rowsum trick).
  - Scores transposed: ST[k,q] via lhsT=KT-block, rhs=QT; two heads packed
    via PE row tiling (K=64 each, partitions 0:64 / 64:128, one XBUS).
  - softmax without max-subtraction (logits ~ N(0,1)); exp on ACT with the
    1/8 scale folded in; causal 0/1 mask multiply post-exp on diagonal
    blocks; fully-masked blocks skipped.
  - AV: lhsT = V-tile [128, 65] (65th col = ones -> row 64 of PSUM is the
    softmax denominator), rhs = P^T tiles.
  - Normalize: psum row 64 -> partition 0 via tiny DMAs, merged
    reciprocal_approx_fast, gpsimd partition_broadcast, DVE multiplies.
  - Output proj from O^T [head*64+d, s] chunks against w_proj rows;
    partial outputs written bf16 (host sums in f32).
"""

import numpy as np
import ml_dtypes

import concourse.bass as bass
import concourse.bacc as bacc
import concourse.mybir as mybir
import concourse.tile as tile
from concourse.bass_utils import run_bass_kernel_spmd

F32 = mybir.dt.float32
BF16 = mybir.dt.bfloat16
EXP = mybir.ActivationFunctionType.Exp

# Problem constants (hardcoded per contract)
B, S, C = 4, 2048, 1024
NH, D = 16, 64
SCALE = D ** -0.5
N_CORES = 8
HG = NH // 2          # heads per core (head group)
NPAIR = HG // 2       # head pairs per core
CCH = C // 128        # contraction chunks for QKV proj
SC = S // 128         # s-chunks (also k-blocks count)
NQT = S // 512        # q-tiles of 512
GW = C // 2           # group width of qkv output (8 heads * 64)


def build_nc():
    nc = bacc.Bacc("TRN2", target_bir_lowering=False, debug=False)

    xT = nc.dram_tensor("xT", [C, S], BF16, kind="ExternalInput")
    wq = nc.dram_tensor("wq", [C, GW], BF16, kind="ExternalInput")
    wk = nc.dram_tensor("wk", [C, GW], BF16, kind="ExternalInput")
    wv = nc.dram_tensor("wv", [C, GW], BF16, kind="ExternalInput")
    wp = nc.dram_tensor("wp", [GW, C], BF16, kind="ExternalInput")
    mask = nc.dram_tensor("mask", [128, 512], BF16, kind="ExternalInput")
    out = nc.dram_tensor("out", [S, C], BF16, kind="ExternalOutput")

    with tile.TileContext(nc) as tc:
        with (
            tc.tile_pool(name="const", bufs=1) as cpool,
            tc.tile_pool(name="pt", bufs=8) as ptpool,
            tc.tile_pool(name="work", bufs=2) as wpool,
            tc.tile_pool(name="ps", bufs=2, space="PSUM") as pspool,
        ):
            # ---- persistent tiles; grouped input DMAs (3D APs) ----
            # chunk cc of a weight lives at cols [512cc:512(cc+1)];
            # chunk cc of xT at cols [2048cc:2048(cc+1)].
            wqall = cpool.tile([128, GW * CCH], BF16, tag="wqall", name="wqall")
            wkall = cpool.tile([128, GW * CCH], BF16, tag="wkall", name="wkall")
            wvall = cpool.tile([128, GW * CCH], BF16, tag="wvall", name="wvall")
            xtall = cpool.tile([128, S * CCH], BF16, tag="xtall", name="xtall")
            mask_sb = cpool.tile([128, 512], BF16, tag="mask", name="maskt")

            # per-chunk DMAs: dram reads stay sequential (a grouped 3D AP
            # with partition-outer ordering turns into 2KB strided bursts
            # at ~51 GB/s -- measured). Arrival order: wq/wk/mask, xT
            # half-A (the st0/st1 ramp chases these), wv, xT half-B, wp.
            # issue from three engine queues in parallel (the ~0.65us
            # per-issue cost on one queue would gate the ramp): wq from
            # scalar, wk from gpsimd, xT/wv/wp/mask from sync
            for cc in range(CCH):
                nc.scalar.dma_start(wqall[:, GW * cc:GW * (cc + 1)],
                                    wq[128 * cc:128 * (cc + 1), :])
            for cc in range(CCH):
                nc.gpsimd.dma_start(wkall[:, GW * cc:GW * (cc + 1)],
                                    wk[128 * cc:128 * (cc + 1), :])
            # full-chunk xT transfers: a column-half split would read
            # every other 2KB of dram at half bandwidth (measured)
            nc.sync.dma_start(xtall[:, 0:S], xT[0:128, :])
            nc.sync.dma_start(mask_sb[:], mask[:, :])
            for cc in range(1, CCH):
                nc.sync.dma_start(xtall[:, S * cc:S * (cc + 1)],
                                  xT[128 * cc:128 * (cc + 1), :])
            for cc in range(CCH):
                nc.sync.dma_start(wvall[:, GW * cc:GW * (cc + 1)],
                                  wv[128 * cc:128 * (cc + 1), :])
            wpall = cpool.tile([128, C * NPAIR], BF16, tag="wpall", name="wpall")
            for p in range(NPAIR):
                nc.sync.dma_start(wpall[:, C * p:C * (p + 1)],
                                  wp[128 * p:128 * (p + 1), :])

            def xt_c(cc):      # xT chunk cc, [128, S]
                return xtall[:, S * cc:S * (cc + 1)]

            def w_c(wall, cc):  # weight chunk cc, [128, GW]
                return wall[:, GW * cc:GW * (cc + 1)]

            # preload the ACT exp table set while input DMAs run
            actwarm = cpool.tile([1, 8], F32, tag="actwarm", name="actwarm")
            nc.vector.memset(actwarm[:], 0.0)
            nc.scalar.activation(actwarm[:], actwarm[:], EXP)

            qt_sb = [cpool.tile([128, S], BF16, tag=f"qt{p}", name=f"qt{p}")
                     for p in range(NPAIR)]
            kt_sb = [cpool.tile([128, S], BF16, tag=f"kt{p}", name=f"kt{p}")
                     for p in range(NPAIR)]
            otn_sb = [cpool.tile([128, S], BF16, tag=f"otn{p}", name=f"otn{p}")
                      for p in range(NPAIR)]
            vt_sb = [cpool.tile([128, 65 * HG], BF16, tag=f"vt{sc}",
                                name=f"vt{sc}")
                     for sc in range(SC)]

            # ---- QT/KT projection unit: one (pair, st, q|k) tile ----
            def emit_qkt_unit(p, st, which):
                ssl = slice(512 * st, 512 * (st + 1))
                wall, dst = (wqall, qt_sb[p]) if which == 0 else \
                    (wkall, kt_sb[p])
                ps = pspool.tile([128, 512], F32, tag="mm", name="qkps",
                                 bufs=2)
                for cc in range(CCH):
                    nc.tensor.matmul(
                        ps[:], w_c(wall, cc)[:, 128 * p:128 * (p + 1)],
                        xt_c(cc)[:, ssl],
                        start=(cc == 0), stop=(cc == CCH - 1))
                nc.vector.tensor_copy(dst[:, ssl], ps[:])

            # ---- pair-0 st0/st1 ramp: tile-minor over 4 accumulators,
            # cc ascending, chasing the chunk DMAs
            ramp = [(0, 0, "sps"), (0, 1, "sps"), (1, 0, "ot"), (1, 1, "ot")]
            pss = [pspool.tile([128, 512], F32, tag=ptag, name="rampps",
                               bufs=2) for _, _, ptag in ramp]
            for cc in range(CCH):
                for (st, which, _), ps in zip(ramp, pss):
                    wall = wqall if which == 0 else wkall
                    nc.tensor.matmul(
                        ps[:], w_c(wall, cc)[:, 0:128],
                        xt_c(cc)[:, 512 * st:512 * (st + 1)],
                        start=(cc == 0), stop=(cc == CCH - 1))
            for (st, which, _), ps in zip(ramp, pss):
                dst = qt_sb[0] if which == 0 else kt_sb[0]
                nc.vector.tensor_copy(dst[:, 512 * st:512 * (st + 1)], ps[:])

            # ---- V strip: V = x @ wv in [s, d] layout + ones column ----
            def emit_v_strip(sc):
                vt = vt_sb[sc]
                nc.gpsimd.memset(vt[:], 1.0)
                ps = pspool.tile([128, GW], F32, tag="mm", name="vps",
                                 bufs=2)
                for cc in range(CCH):
                    nc.tensor.matmul(
                        ps[:], xt_c(cc)[:, 128 * sc:128 * (sc + 1)],
                        w_c(wvall, cc)[:],
                        start=(cc == 0), stop=(cc == CCH - 1))
                vt_v = vt[:, :].rearrange("p (h d) -> p h d", h=HG)[:, :, 0:64]
                ps_v = ps[:, :].rearrange("p (h d) -> p h d", h=HG)
                nc.vector.tensor_copy(vt_v, ps_v)



            # ---- out-proj chunk: out[s-chunk,:] = sum_p OTn_p.T @ wp_p ----
            def emit_out_chunk(sc):
                outst = wpool.tile([128, C], BF16, tag="outst", name="outst")
                for half in range(2):
                    pp = pspool.tile([128, 512], F32, tag="mm", name="pp",
                                     bufs=2)
                    for p in range(NPAIR):
                        nc.tensor.matmul(
                            pp[:], otn_sb[p][:, 128 * sc:128 * (sc + 1)],
                            wpall[:, C * p + 512 * half:
                                  C * p + 512 * (half + 1)],
                            start=(p == 0), stop=(p == NPAIR - 1))
                    nc.vector.tensor_copy(
                        outst[:, 512 * half:512 * (half + 1)], pp[:])
                nc.sync.dma_start(out[128 * sc:128 * (sc + 1), :], outst[:])

            # ---- per head-pair attention, k-block granular ----
            deferred_norm = [None]  # phase-2 closure from the previous j

            def run_deferred():
                if deferred_norm[0] is not None:
                    deferred_norm[0]()
                    deferred_norm[0] = None

            for p in range(NPAIR):
                qt, kt = qt_sb[p], kt_sb[p]
                for j in range(NQT):
                    nkb = 4 * (j + 1)  # causal: only k-blocks 0..nkb-1
                    # Filler units for this (p, j) window, pinned to slots
                    # so ACT-light q-tiles aren't overloaded. V strips for
                    # q-tile j's own NEW k-blocks can live inside j itself
                    # (the AV flush at j-end is their first consumer).
                    fill_at = {}

                    def add_at(slot, fn, fill_at=fill_at, nkb=nkb):
                        fill_at.setdefault(min(slot, nkb - 1), []).append(fn)

                    if p == 0:
                        for i, sc in enumerate(range(4 * j, 4 * j + 4)):
                            add_at((0, 0, 1, 2)[i],
                                   lambda sc=sc: emit_v_strip(sc))
                        if j == 1:
                            for w in range(2):
                                add_at(5 + 2 * w, lambda w=w:
                                       emit_qkt_unit(0, 2, w))
                        if j == 2:
                            for w in range(2):
                                add_at(5 + 2 * w, lambda w=w:
                                       emit_qkt_unit(0, 3, w))
                        if j == 3:
                            for w in range(2):
                                add_at(2 + 4 * w, lambda w=w:
                                       emit_qkt_unit(1, 0, w))
                            for w in range(2):
                                add_at(10 + 4 * w, lambda w=w:
                                       emit_qkt_unit(1, 1, w))
                    else:
                        if j == 0:
                            for w in range(2):
                                add_at(0 + 2 * w, lambda w=w, pp_=p:
                                       emit_qkt_unit(pp_, 2, w))
                        if j == 1:
                            for w in range(2):
                                add_at(0 + 4 * w, lambda w=w, pp_=p:
                                       emit_qkt_unit(pp_, 3, w))
                        if p < 3:
                            if j == 2:
                                for w in range(2):
                                    add_at(0 + 6 * w, lambda w=w, pp_=p + 1:
                                           emit_qkt_unit(pp_, 0, w))
                            if j == 3:
                                for w in range(2):
                                    add_at(0 + 8 * w, lambda w=w, pp_=p + 1:
                                           emit_qkt_unit(pp_, 1, w))
                    if p == 3 and j >= 1:
                        # out-proj chunks of q-tile j-1, spread from slot 5
                        # (just after the deferred normalize at slot 4)
                        # to the end of the window
                        for i, sc in enumerate(range(4 * (j - 1), 4 * j)):
                            add_at(5 + (i * (nkb - 6)) // 3,
                                   lambda sc=sc: emit_out_chunk(sc))

                    ot = [pspool.tile([65, 512], F32, tag="ot", name="ot",
                                      bufs=2) for _ in range(2)]

                    def emit_av(kb, pt, j=j, nkb=nkb, ot=ot, p=p):
                        o = 128 * (kb - 4 * j) if kb >= 4 * j else 0
                        for h in range(2):
                            nc.tensor.matmul(
                                ot[h][:, o:512],
                                vt_sb[kb][:, 65 * (2 * p + h):
                                          65 * (2 * p + h) + 65],
                                pt[:, 512 * h + o:512 * (h + 1)],
                                start=(kb == 0), stop=(kb == nkb - 1))

                    pending = []
                    sp_of = {}
                    for kb in range(nkb):
                        diag = kb >= 4 * j
                        o = 128 * (kb - 4 * j) if diag else 0
                        # scores (transposed), both heads packed side by
                        # side in one psum tile: h at cols [512h+o:512h+512].
                        # Both k-blocks of a pair are emitted back-to-back
                        # (before the exps/AVs) so 3 of the 4 score
                        # LDWEIGHTS pull ahead into in-flight matmuls.
                        if kb % 2 == 0:
                            for kb2 in (kb, kb + 1):
                                o2 = 128 * (kb2 - 4 * j) if kb2 >= 4 * j \
                                    else 0
                                sp2 = pspool.tile([128, 1024], F32,
                                                  tag="sps", name="sps",
                                                  bufs=2)
                                for h in range(2):
                                    hsl = slice(64 * h, 64 * (h + 1))
                                    nc.tensor.matmul(
                                        sp2[:, 512 * h + o2:512 * (h + 1)],
                                        kt[hsl, 128 * kb2:128 * (kb2 + 1)],
                                        qt[hsl, 512 * j + o2:512 * (j + 1)],
                                        start=True, stop=True)
                                sp_of[kb2] = sp2
                        sp = sp_of[kb]
                        pt = ptpool.tile([128, 1024], BF16, tag="pt",
                                         name="pt")
                        if o > 0:
                            # one ACTIVATE over both heads' live columns
                            spv = sp[:, :].rearrange(
                                "p (h f) -> p h f", h=2)[:, :, o:512]
                            ptv = pt[:, :].rearrange(
                                "p (h f) -> p h f", h=2)[:, :, o:512]
                            nc.scalar.activation(ptv, spv, EXP, scale=SCALE)
                        else:
                            nc.scalar.activation(pt[:], sp[:], EXP,
                                                 scale=SCALE)
                        if diag:
                            for h in range(2):
                                csl = slice(512 * h + o, 512 * (h + 1))
                                nc.vector.tensor_mul(
                                    pt[:, csl], pt[:, csl],
                                    mask_sb[:, 0:512 - o])
                        pending.append((kb, pt))
                        avlag = 2 if (p == 3 and j == 3 and kb >= 12) else 3
                        if len(pending) > avlag:
                            emit_av(*pending.pop(0))
                        if kb == (4 if nkb > 4 else nkb - 1):
                            run_deferred()
                        for f in fill_at.get(kb, []):
                            f()
                    for item in pending:
                        emit_av(*item)

                    # ---- normalize phase 1: denominators (psum row 64)
                    # straight to partition 0 via tiny gpsimd-issued DMAs,
                    # bf16 numerator casts release the psum banks, merged
                    # reciprocal, one merged bf16 broadcast.
                    qsl = slice(512 * j, 512 * (j + 1))
                    s64a = wpool.tile([65, 512], F32, tag="s64a", name="s64a")
                    s64b = wpool.tile([65, 512], F32, tag="s64b", name="s64b")
                    nc.vector.tensor_copy(s64a[:, :], ot[0][:, :])
                    nc.vector.tensor_copy(s64b[:, :], ot[1][:, :])
                    dens = wpool.tile([1, 1024], F32, tag="dens", name="dens")
                    nc.gpsimd.dma_start(dens[0:1, 0:512], s64a[64:65, :])
                    nc.gpsimd.dma_start(dens[0:1, 512:1024], s64b[64:65, :])
                    inv = wpool.tile([1, 1024], F32, tag="inv", name="inv")
                    nc.vector.reciprocal_approx_fast(inv[0:1, :],
                                                     dens[0:1, :])
                    bcs = wpool.tile([64, 1024], F32, tag="bcs", name="bcs")
                    nc.gpsimd.partition_broadcast(bcs[:], inv[0:1, :])

                    # ---- phase 2 (deferred into the next q-tile, after
                    # the broadcast has finished): multiplies on DVE,
                    # partition-shifting oth copy from the gpsimd queue.
                    # The very last one (pair 3, q-tile 3) is split per
                    # s-chunk so each tail out-proj chunk can start as
                    # soon as its own 128 columns are normalized.
                    last = (p == 3 and j == 3)

                    def phase2(p=p, qsl=qsl, s64a=s64a, s64b=s64b, bcs=bcs,
                               last=last):
                        oth = wpool.tile([64, 512], BF16, tag="oth",
                                         name="oth")
                        nsplit = 4 if last else 1
                        w = 512 // nsplit
                        for i in range(nsplit):
                            csl = slice(w * i, w * (i + 1))
                            osl = slice(qsl.start + w * i,
                                        qsl.start + w * (i + 1))
                            nc.vector.tensor_mul(otn_sb[p][0:64, osl],
                                                 s64a[0:64, csl],
                                                 bcs[:, csl])
                            nc.vector.tensor_mul(oth[:, csl],
                                                 s64b[0:64, csl],
                                                 bcs[:, 512:1024][:, csl])
                            nc.gpsimd.dma_start(otn_sb[p][64:128, osl],
                                                oth[:, csl])

                    deferred_norm[0] = phase2

            # ---- tail: chunks 12-14 pre-accumulate pairs 0-2 while the
            # last normalize chain (otn[3] q-tile 3) is still in flight,
            # then finish with pair 3's contribution; chunk 15 runs whole.
            tail_pps = {}
            for sc, ptag in ((12, "mm"), (13, "sps"), (14, "ot"), (15, "mm")):
                pps = []
                for half in range(2):
                    pp = pspool.tile([128, 512], F32, tag=ptag, name="pp",
                                     bufs=2)
                    for p in range(3):
                        nc.tensor.matmul(
                            pp[:], otn_sb[p][:, 128 * sc:128 * (sc + 1)],
                            wpall[:, C * p + 512 * half:
                                  C * p + 512 * (half + 1)],
                            start=(p == 0), stop=False)
                    pps.append(pp)
                tail_pps[sc] = pps
            run_deferred()
            for sc in (12, 13, 14, 15):
                outst = wpool.tile([128, C], BF16, tag="outst", name="outst")
                for half, pp in enumerate(tail_pps[sc]):
                    nc.tensor.matmul(
                        pp[:], otn_sb[3][:, 128 * sc:128 * (sc + 1)],
                        wpall[:, C * 3 + 512 * half:C * 3 + 512 * (half + 1)],
                        start=False, stop=True)
                    nc.vector.tensor_copy(
                        outst[:, 512 * half:512 * (half + 1)], pp[:])
                nc.sync.dma_start(out[128 * sc:128 * (sc + 1), :], outst[:])

    nc.compile()
    return nc


_NC_CACHE = None


def _get_nc():
    global _NC_CACHE
    if _NC_CACHE is None:
        _NC_CACHE = build_nc()
    return _NC_CACHE


def make_in_maps(x, w_qkv, w_proj):
    """Shard full inputs into the 8 per-core input dicts."""
    bf = ml_dtypes.bfloat16
    mask01 = (np.arange(128)[:, None] <= np.arange(512)[None, :]) \
        .astype(bf)
    in_maps = []
    for core in range(N_CORES):
        b, g = core // 2, core % 2
        gsl = slice(GW * g, GW * (g + 1))
        in_maps.append({
            "xT": np.ascontiguousarray(x[b].T).astype(bf),
            "wq": np.ascontiguousarray(w_qkv[:, 0 * C:1 * C][:, gsl]).astype(bf),
            "wk": np.ascontiguousarray(w_qkv[:, 1 * C:2 * C][:, gsl]).astype(bf),
            "wv": np.ascontiguousarray(w_qkv[:, 2 * C:3 * C][:, gsl]).astype(bf),
            "wp": np.ascontiguousarray(w_proj[gsl, :]).astype(bf),
            "mask": mask01,
        })
    return in_maps


def kernel(x, w_qkv, w_proj, b_proj, _profile=False):
    import os
    if not _profile:
        # the NTFF trace path needs modules absent from this image;
        # make sure an inherited BASS_TRACE can't route us into it
        os.environ["BASS_NEVER_TRACE"] = "1"
    else:
        os.environ.pop("BASS_NEVER_TRACE", None)
    x = np.asarray(x, np.float32)
    w_qkv = np.asarray(w_qkv, np.float32)
    w_proj = np.asarray(w_proj, np.float32)
    b_proj = np.asarray(b_proj, np.float32)

    nc = _get_nc()
    in_maps = make_in_maps(x, w_qkv, w_proj)
    res = run_bass_kernel_spmd(nc, in_maps, core_ids=list(range(N_CORES)),
                               trace=_profile)
    partials = [np.asarray(res.results[c]["out"], np.float32)
                for c in range(N_CORES)]
    out = np.empty((B, S, C), np.float32)
    for b in range(B):
        out[b] = partials[2 * b] + partials[2 * b + 1] + b_proj
    if _profile:
        return out, res
    return out


# revision 35
# speedup vs baseline: 1.0097x; 1.0097x over previous
"""Causal multi-head flash-attention block (QKV proj + attention + out proj)
for Trainium2, distributed over 8 NeuronCores.

Sharding: data-parallel over batch (B=4) x tensor-parallel over head groups
(16 heads -> 2 groups of 8). Core c handles batch c//2, head group c%2.
Each core computes a partial output projection (its 8 heads' contribution);
the host sums the two partials per batch and adds the bias.

v3 schedule notes: the attention inner loop is ACT(exp)-bound, and the PE
executes its queue strictly in order, so independent matmul work is
interleaved INTO the attention k-block loop to keep the PE dense:
  - each pair's (st2,st3) QT/KT units fill its own j0/j1, the NEXT pair's
    (st0,st1) units fill j2/j3 (so casts land well before the pair switch),
  - V strips fill pair 0, out-proj chunks fill pair 3's late slots,
  - inputs arrive via 13 grouped multi-chunk DMAs (3D access patterns) so
    the sync queue isn't issue-bound; pair-0 st0/st1 QT/KT runs tile-minor
    over the arriving chunk-pairs (PE dense from ~11us).
Scores/exp are k-block granular: one ACTIVATE per k-block covering both
packed heads, restricted to causally live columns on diagonal blocks.
The softmax-normalize chain is split in two phases and issues its DMAs
from the producer engines' own queues (dens from vector, the oth
partition-shift from gpsimd) so no FIFO head-of-line-blocks another
engine's PE-feeding work; the multiplies run late (deferred one q-tile)
on the DVE after the gpsimd broadcast is long done.

Per-core kernel (all matmuls bf16 operands, fp32 PSUM accumulate):
  - QKV proj from host-pretransposed x^T: Q^T,K^T in [d, s] layout, V in
    [s, d] layout with a ones-column per head (rowsum trick).
  - Scores transposed: ST[k,q] via lhsT=KT-block, rhs=QT; two heads packed
    via PE row tiling (K=64 each, partitions 0:64 / 64:128, one XBUS).
  - softmax without max-subtraction (logits ~ N(0,1)); exp on ACT with the
    1/8 scale folded in; causal 0/1 mask multiply post-exp on diagonal
    blocks; fully-masked blocks skipped.
  - AV: lhsT = V-tile [128, 65] (65th col = ones -> row 64 of PSUM is the
    softmax denominator), rhs = P^T tiles.
  - Normalize: psum row 64 -> partition 0 via tiny DMAs, merged
    reciprocal_approx_fast, gpsimd partition_broadcast, DVE multiplies.
  - Output proj from O^T [head*64+d, s] chunks against w_proj rows;
    partial outputs written bf16 (host sums in f32).
"""

import numpy as np
import ml_dtypes

import concourse.bass as bass
import concourse.bacc as bacc
import concourse.mybir as mybir
import concourse.tile as tile
from concourse.bass_utils import run_bass_kernel_spmd

F32 = mybir.dt.float32
BF16 = mybir.dt.bfloat16
EXP = mybir.ActivationFunctionType.Exp

# Problem constants (hardcoded per contract)
B, S, C = 4, 2048, 1024
NH, D = 16, 64
SCALE = D ** -0.5
N_CORES = 8
HG = NH // 2          # heads per core (head group)
NPAIR = HG // 2       # head pairs per core
CCH = C // 128        # contraction chunks for QKV proj
SC = S // 128         # s-chunks (also k-blocks count)
NQT = S // 512        # q-tiles of 512
GW = C // 2           # group width of qkv output (8 heads * 64)


def build_nc():
    nc = bacc.Bacc("TRN2", target_bir_lowering=False, debug=False)

    xT = nc.dram_tensor("xT", [C, S], BF16, kind="ExternalInput")
    wq = nc.dram_tensor("wq", [C, GW], BF16, kind="ExternalInput")
    wk = nc.dram_tensor("wk", [C, GW], BF16, kind="ExternalInput")
    wv = nc.dram_tensor("wv", [C, GW], BF16, kind="ExternalInput")
    wp = nc.dram_tensor("wp", [GW, C], BF16, kind="ExternalInput")
    mask = nc.dram_tensor("mask", [128, 512], BF16, kind="ExternalInput")
    out = nc.dram_tensor("out", [S, C], BF16, kind="ExternalOutput")

    with tile.TileContext(nc) as tc:
        with (
            tc.tile_pool(name="const", bufs=1) as cpool,
            tc.tile_pool(name="pt", bufs=8) as ptpool,
            tc.tile_pool(name="work", bufs=2) as wpool,
            tc.tile_pool(name="ps", bufs=2, space="PSUM") as pspool,
        ):
            # ---- persistent tiles; grouped input DMAs (3D APs) ----
            # chunk cc of a weight lives at cols [512cc:512(cc+1)];
            # chunk cc of xT at cols [2048cc:2048(cc+1)].
            wqall = cpool.tile([128, GW * CCH], BF16, tag="wqall", name="wqall")
            wkall = cpool.tile([128, GW * CCH], BF16, tag="wkall", name="wkall")
            wvall = cpool.tile([128, GW * CCH], BF16, tag="wvall", name="wvall")
            xtall = cpool.tile([128, S * CCH], BF16, tag="xtall", name="xtall")
            mask_sb = cpool.tile([128, 512], BF16, tag="mask", name="maskt")

            # per-chunk DMAs: dram reads stay sequential (a grouped 3D AP
            # with partition-outer ordering turns into 2KB strided bursts
            # at ~51 GB/s -- measured). Arrival order: wq/wk/mask, xT
            # half-A (the st0/st1 ramp chases these), wv, xT half-B, wp.
            # issue from three engine queues in parallel (the ~0.65us
            # per-issue cost on one queue would gate the ramp): wq from
            # scalar, wk from gpsimd, xT/wv/wp/mask from sync
            for cc in range(CCH):
                nc.scalar.dma_start(wqall[:, GW * cc:GW * (cc + 1)],
                                    wq[128 * cc:128 * (cc + 1), :])
            for cc in range(CCH):
                nc.gpsimd.dma_start(wkall[:, GW * cc:GW * (cc + 1)],
                                    wk[128 * cc:128 * (cc + 1), :])
            # full-chunk xT transfers: a column-half split would read
            # every other 2KB of dram at half bandwidth (measured)
            nc.sync.dma_start(xtall[:, 0:S], xT[0:128, :])
            nc.sync.dma_start(mask_sb[:], mask[:, :])
            for cc in range(1, CCH):
                nc.sync.dma_start(xtall[:, S * cc:S * (cc + 1)],
                                  xT[128 * cc:128 * (cc + 1), :])
            for cc in range(CCH):
                nc.sync.dma_start(wvall[:, GW * cc:GW * (cc + 1)],
                                  wv[128 * cc:128 * (cc + 1), :])
            wpall = cpool.tile([128, C * NPAIR], BF16, tag="wpall", name="wpall")
            for p in range(NPAIR):
                nc.sync.dma_start(wpall[:, C * p:C * (p + 1)],
                                  wp[128 * p:128 * (p + 1), :])

            def xt_c(cc):      # xT chunk cc, [128, S]
                return xtall[:, S * cc:S * (cc + 1)]

            def w_c(wall, cc):  # weight chunk cc, [128, GW]
                return wall[:, GW * cc:GW * (cc + 1)]

            # preload the ACT exp table set while input DMAs run
            actwarm = cpool.tile([1, 8], F32, tag="actwarm", name="actwarm")
            nc.vector.memset(actwarm[:], 0.0)
            nc.scalar.activation(actwarm[:], actwarm[:], EXP)

            qt_sb = [cpool.tile([128, S], BF16, tag=f"qt{p}", name=f"qt{p}")
                     for p in range(NPAIR)]
            kt_sb = [cpool.tile([128, S], BF16, tag=f"kt{p}", name=f"kt{p}")
                     for p in range(NPAIR)]
            otn_sb = [cpool.tile([128, S], BF16, tag=f"otn{p}", name=f"otn{p}")
                      for p in range(NPAIR)]
            vt_sb = [cpool.tile([128, 65 * HG], BF16, tag=f"vt{sc}",
                                name=f"vt{sc}")
                     for sc in range(SC)]

            # ---- QT/KT projection unit: one (pair, st, q|k) tile ----
            def emit_qkt_unit(p, st, which):
                ssl = slice(512 * st, 512 * (st + 1))
                wall, dst = (wqall, qt_sb[p]) if which == 0 else \
                    (wkall, kt_sb[p])
                ps = pspool.tile([128, 512], F32, tag="mm", name="qkps",
                                 bufs=2)
                for cc in range(CCH):
                    nc.tensor.matmul(
                        ps[:], w_c(wall, cc)[:, 128 * p:128 * (p + 1)],
                        xt_c(cc)[:, ssl],
                        start=(cc == 0), stop=(cc == CCH - 1))
                nc.vector.tensor_copy(dst[:, ssl], ps[:])

            # ---- pair-0 st0/st1 ramp: tile-minor over 4 accumulators,
            # cc ascending, chasing the chunk DMAs
            ramp = [(0, 0, "sps"), (0, 1, "sps"), (1, 0, "ot"), (1, 1, "ot")]
            pss = [pspool.tile([128, 512], F32, tag=ptag, name="rampps",
                               bufs=2) for _, _, ptag in ramp]
            for cc in range(CCH):
                for (st, which, _), ps in zip(ramp, pss):
                    wall = wqall if which == 0 else wkall
                    nc.tensor.matmul(
                        ps[:], w_c(wall, cc)[:, 0:128],
                        xt_c(cc)[:, 512 * st:512 * (st + 1)],
                        start=(cc == 0), stop=(cc == CCH - 1))
            for (st, which, _), ps in zip(ramp, pss):
                dst = qt_sb[0] if which == 0 else kt_sb[0]
                nc.vector.tensor_copy(dst[:, 512 * st:512 * (st + 1)], ps[:])

            # ---- V strip: V = x @ wv in [s, d] layout + ones column ----
            def emit_v_strip(sc):
                vt = vt_sb[sc]
                nc.gpsimd.memset(vt[:], 1.0)
                ps = pspool.tile([128, GW], F32, tag="mm", name="vps",
                                 bufs=2)
                for cc in range(CCH):
                    nc.tensor.matmul(
                        ps[:], xt_c(cc)[:, 128 * sc:128 * (sc + 1)],
                        w_c(wvall, cc)[:],
                        start=(cc == 0), stop=(cc == CCH - 1))
                vt_v = vt[:, :].rearrange("p (h d) -> p h d", h=HG)[:, :, 0:64]
                ps_v = ps[:, :].rearrange("p (h d) -> p h d", h=HG)
                nc.vector.tensor_copy(vt_v, ps_v)



            # ---- out-proj chunk: out[s-chunk,:] = sum_p OTn_p.T @ wp_p ----
            def emit_out_chunk(sc):
                outst = wpool.tile([128, C], BF16, tag="outst", name="outst")
                for half in range(2):
                    pp = pspool.tile([128, 512], F32, tag="mm", name="pp",
                                     bufs=2)
                    for p in range(NPAIR):
                        nc.tensor.matmul(
                            pp[:], otn_sb[p][:, 128 * sc:128 * (sc + 1)],
                            wpall[:, C * p + 512 * half:
                                  C * p + 512 * (half + 1)],
                            start=(p == 0), stop=(p == NPAIR - 1))
                    nc.vector.tensor_copy(
                        outst[:, 512 * half:512 * (half + 1)], pp[:])
                nc.sync.dma_start(out[128 * sc:128 * (sc + 1), :], outst[:])

            # ---- per head-pair attention, k-block granular ----
            deferred_norm = [None]  # phase-2 closure from the previous j

            def run_deferred():
                if deferred_norm[0] is not None:
                    deferred_norm[0]()
                    deferred_norm[0] = None

            for p in range(NPAIR):
                qt, kt = qt_sb[p], kt_sb[p]
                for j in range(NQT):
                    nkb = 4 * (j + 1)  # causal: only k-blocks 0..nkb-1
                    # Filler units for this (p, j) window, pinned to slots
                    # so ACT-light q-tiles aren't overloaded. V strips for
                    # q-tile j's own NEW k-blocks can live inside j itself
                    # (the AV flush at j-end is their first consumer).
                    fill_at = {}

                    def add_at(slot, fn, fill_at=fill_at, nkb=nkb):
                        fill_at.setdefault(min(slot, nkb - 1), []).append(fn)

                    if p == 0:
                        for i, sc in enumerate(range(4 * j, 4 * j + 4)):
                            add_at((0, 0, 1, 2)[i],
                                   lambda sc=sc: emit_v_strip(sc))
                        if j == 1:
                            for w in range(2):
                                add_at(5 + 2 * w, lambda w=w:
                                       emit_qkt_unit(0, 2, w))
                        if j == 2:
                            for w in range(2):
                                add_at(5 + 2 * w, lambda w=w:
                                       emit_qkt_unit(0, 3, w))
                        if j == 3:
                            for w in range(2):
                                add_at(2 + 4 * w, lambda w=w:
                                       emit_qkt_unit(1, 0, w))
                            for w in range(2):
                                add_at(10 + 4 * w, lambda w=w:
                                       emit_qkt_unit(1, 1, w))
                    else:
                        if j == 0:
                            for w in range(2):
                                add_at(0 + 2 * w, lambda w=w, pp_=p:
                                       emit_qkt_unit(pp_, 2, w))
                        if j == 1:
                            for w in range(2):
                                add_at(0 + 4 * w, lambda w=w, pp_=p:
                                       emit_qkt_unit(pp_, 3, w))
                        if p < 3:
                            if j == 2:
                                for w in range(2):
                                    add_at(0 + 6 * w, lambda w=w, pp_=p + 1:
                                           emit_qkt_unit(pp_, 0, w))
                            if j == 3:
                                for w in range(2):
                                    add_at(0 + 8 * w, lambda w=w, pp_=p + 1:
                                           emit_qkt_unit(pp_, 1, w))
                    if p == 3 and j >= 1:
                        # out-proj chunks of q-tile j-1 in the second half
                        # (their otn inputs come from the previous
                        # normalize; placing them early would stall the
                        # in-order PE)
                        for i, sc in enumerate(range(4 * (j - 1), 4 * j)):
                            add_at(nkb - 4 + i,
                                   lambda sc=sc: emit_out_chunk(sc))

                    ot = [pspool.tile([65, 512], F32, tag="ot", name="ot",
                                      bufs=2) for _ in range(2)]

                    def emit_av(kb, pt, j=j, nkb=nkb, ot=ot, p=p):
                        o = 128 * (kb - 4 * j) if kb >= 4 * j else 0
                        for h in range(2):
                            nc.tensor.matmul(
                                ot[h][:, o:512],
                                vt_sb[kb][:, 65 * (2 * p + h):
                                          65 * (2 * p + h) + 65],
                                pt[:, 512 * h + o:512 * (h + 1)],
                                start=(kb == 0), stop=(kb == nkb - 1))

                    pending = []
                    sp_of = {}
                    for kb in range(nkb):
                        diag = kb >= 4 * j
                        o = 128 * (kb - 4 * j) if diag else 0
                        # scores (transposed), both heads packed side by
                        # side in one psum tile: h at cols [512h+o:512h+512].
                        # Both k-blocks of a pair are emitted back-to-back
                        # (before the exps/AVs) so 3 of the 4 score
                        # LDWEIGHTS pull ahead into in-flight matmuls.
                        if kb % 2 == 0:
                            for kb2 in (kb, kb + 1):
                                o2 = 128 * (kb2 - 4 * j) if kb2 >= 4 * j \
                                    else 0
                                sp2 = pspool.tile([128, 1024], F32,
                                                  tag="sps", name="sps",
                                                  bufs=2)
                                for h in range(2):
                                    hsl = slice(64 * h, 64 * (h + 1))
                                    nc.tensor.matmul(
                                        sp2[:, 512 * h + o2:512 * (h + 1)],
                                        kt[hsl, 128 * kb2:128 * (kb2 + 1)],
                                        qt[hsl, 512 * j + o2:512 * (j + 1)],
                                        start=True, stop=True)
                                sp_of[kb2] = sp2
                        sp = sp_of[kb]
                        pt = ptpool.tile([128, 1024], BF16, tag="pt",
                                         name="pt")
                        if o > 0:
                            # one ACTIVATE over both heads' live columns
                            spv = sp[:, :].rearrange(
                                "p (h f) -> p h f", h=2)[:, :, o:512]
                            ptv = pt[:, :].rearrange(
                                "p (h f) -> p h f", h=2)[:, :, o:512]
                            nc.scalar.activation(ptv, spv, EXP, scale=SCALE)
                        else:
                            nc.scalar.activation(pt[:], sp[:], EXP,
                                                 scale=SCALE)
                        if diag:
                            for h in range(2):
                                csl = slice(512 * h + o, 512 * (h + 1))
                                nc.vector.tensor_mul(
                                    pt[:, csl], pt[:, csl],
                                    mask_sb[:, 0:512 - o])
                        pending.append((kb, pt))
                        avlag = 2 if (p == 3 and j == 3 and kb >= 12) else 4
                        if len(pending) > avlag:
                            emit_av(*pending.pop(0))
                        if kb == (4 if nkb > 4 else nkb - 1):
                            run_deferred()
                        for f in fill_at.get(kb, []):
                            f()
                    for item in pending:
                        emit_av(*item)

                    # ---- normalize phase 1: denominators (psum row 64)
                    # straight to partition 0 via tiny gpsimd-issued DMAs,
                    # bf16 numerator casts release the psum banks, merged
                    # reciprocal, one merged bf16 broadcast.
                    qsl = slice(512 * j, 512 * (j + 1))
                    s64a = wpool.tile([65, 512], F32, tag="s64a", name="s64a")
                    s64b = wpool.tile([65, 512], F32, tag="s64b", name="s64b")
                    nc.vector.tensor_copy(s64a[:, :], ot[0][:, :])
                    nc.vector.tensor_copy(s64b[:, :], ot[1][:, :])
                    dens = wpool.tile([1, 1024], F32, tag="dens", name="dens")
                    nc.gpsimd.dma_start(dens[0:1, 0:512], s64a[64:65, :])
                    nc.gpsimd.dma_start(dens[0:1, 512:1024], s64b[64:65, :])
                    inv = wpool.tile([1, 1024], F32, tag="inv", name="inv")
                    nc.vector.reciprocal_approx_fast(inv[0:1, :],
                                                     dens[0:1, :])
                    bcs = wpool.tile([64, 1024], F32, tag="bcs", name="bcs")
                    nc.gpsimd.partition_broadcast(bcs[:], inv[0:1, :])

                    # ---- phase 2 (deferred into the next q-tile, after
                    # the broadcast has finished): multiplies on DVE,
                    # partition-shifting oth copy from the gpsimd queue.
                    # The very last one (pair 3, q-tile 3) is split per
                    # s-chunk so each tail out-proj chunk can start as
                    # soon as its own 128 columns are normalized.
                    last = (p == 3 and j == 3)

                    def phase2(p=p, qsl=qsl, s64a=s64a, s64b=s64b, bcs=bcs,
                               last=last):
                        oth = wpool.tile([64, 512], BF16, tag="oth",
                                         name="oth")
                        nsplit = 4 if last else 1
                        w = 512 // nsplit
                        for i in range(nsplit):
                            csl = slice(w * i, w * (i + 1))
                            osl = slice(qsl.start + w * i,
                                        qsl.start + w * (i + 1))
                            nc.vector.tensor_mul(otn_sb[p][0:64, osl],
                                                 s64a[0:64, csl],
                                                 bcs[:, csl])
                            nc.vector.tensor_mul(oth[:, csl],
                                                 s64b[0:64, csl],
                                                 bcs[:, 512:1024][:, csl])
                            nc.gpsimd.dma_start(otn_sb[p][64:128, osl],
                                                oth[:, csl])

                    deferred_norm[0] = phase2

            # ---- tail: chunks 12-14 pre-accumulate pairs 0-2 while the
            # last normalize chain (otn[3] q-tile 3) is still in flight,
            # then finish with pair 3's contribution; chunk 15 runs whole.
            tail_pps = {}
            for sc, ptag in ((12, "mm"), (13, "sps"), (14, "ot"), (15, "mm")):
                pps = []
                for half in range(2):
                    pp = pspool.tile([128, 512], F32, tag=ptag, name="pp",
                                     bufs=2)
                    for p in range(3):
                        nc.tensor.matmul(
                            pp[:], otn_sb[p][:, 128 * sc:128 * (sc + 1)],
                            wpall[:, C * p + 512 * half:
                                  C * p + 512 * (half + 1)],
                            start=(p == 0), stop=False)
                    pps.append(pp)
                tail_pps[sc] = pps
            run_deferred()
            for sc in (12, 13, 14, 15):
                outst = wpool.tile([128, C], BF16, tag="outst", name="outst")
                for half, pp in enumerate(tail_pps[sc]):
                    nc.tensor.matmul(
                        pp[:], otn_sb[3][:, 128 * sc:128 * (sc + 1)],
                        wpall[:, C * 3 + 512 * half:C * 3 + 512 * (half + 1)],
                        start=False, stop=True)
                    nc.vector.tensor_copy(
                        outst[:, 512 * half:512 * (half + 1)], pp[:])
                nc.sync.dma_start(out[128 * sc:128 * (sc + 1), :], outst[:])

    nc.compile()
    return nc


_NC_CACHE = None


def _get_nc():
    global _NC_CACHE
    if _NC_CACHE is None:
        _NC_CACHE = build_nc()
    return _NC_CACHE


def make_in_maps(x, w_qkv, w_proj):
    """Shard full inputs into the 8 per-core input dicts."""
    bf = ml_dtypes.bfloat16
    mask01 = (np.arange(128)[:, None] <= np.arange(512)[None, :]) \
        .astype(bf)
    in_maps = []
    for core in range(N_CORES):
        b, g = core // 2, core % 2
        gsl = slice(GW * g, GW * (g + 1))
        in_maps.append({
            "xT": np.ascontiguousarray(x[b].T).astype(bf),
            "wq": np.ascontiguousarray(w_qkv[:, 0 * C:1 * C][:, gsl]).astype(bf),
            "wk": np.ascontiguousarray(w_qkv[:, 1 * C:2 * C][:, gsl]).astype(bf),
            "wv": np.ascontiguousarray(w_qkv[:, 2 * C:3 * C][:, gsl]).astype(bf),
            "wp": np.ascontiguousarray(w_proj[gsl, :]).astype(bf),
            "mask": mask01,
        })
    return in_maps


def kernel(x, w_qkv, w_proj, b_proj, _profile=False):
    import os
    if not _profile:
        # the NTFF trace path needs modules absent from this image;
        # make sure an inherited BASS_TRACE can't route us into it
        os.environ["BASS_NEVER_TRACE"] = "1"
    else:
        os.environ.pop("BASS_NEVER_TRACE", None)
    x = np.asarray(x, np.float32)
    w_qkv = np.asarray(w_qkv, np.float32)
    w_proj = np.asarray(w_proj, np.float32)
    b_proj = np.asarray(b_proj, np.float32)

    nc = _get_nc()
    in_maps = make_in_maps(x, w_qkv, w_proj)
    res = run_bass_kernel_spmd(nc, in_maps, core_ids=list(range(N_CORES)),
                               trace=_profile)
    partials = [np.asarray(res.results[c]["out"], np.float32)
                for c in range(N_CORES)]
    out = np.empty((B, S, C), np.float32)
    for b in range(B):
        out[b] = partials[2 * b] + partials[2 * b + 1] + b_proj
    if _profile:
        return out, res
    return out
